# revision 7
# baseline (speedup 1.0000x reference)
"""Trainium2 Bass kernel for the 2-layer-LSTM tactile rollout (nn_ACTP).

Strategy: data-parallel over batch (B=512 -> 64 rows/core on 8 cores).
All weights stay SBUF-resident (f32r, DMA'd directly). LSTM + FC1
matmuls run batch-major (activations stationary, weights moving,
float32r at N=512 -> full PE rate); FC2 runs feature-major (weights
stationary) so the rollout feedback path needs no extra transpose.
Gate order is reshuffled to [g i f o]; each gate gets its own [64,512]
PSUM bank so activations consume chunks as the PE closes them and the
next step's recurrence matmuls start early. Gate/FC1 biases ride a
ones-row in the stationary operand; the FC2 bias uses the ACT
per-partition bias port. h1/h2/out3 are re-transposed each step via
PE-transpose (4x [64,128] each). mae partials exit as a [64,1]
per-core column, reduced on host.
"""
import sys
import types

sys.path.insert(0, "/opt/trn_rl_repo")

import numpy as np

# ---- NTFF profile hook (missing antenv.axon_hooks in this image) ----
try:
    import antenv
    if "antenv.axon_hooks" not in sys.modules:
        _mod = types.ModuleType("antenv.axon_hooks")
        _hooks = [None]
        _mod.set_axon_ntff_profile_hook = lambda h: _hooks.__setitem__(0, h)
        _mod.get_axon_ntff_profile_hook = lambda: _hooks[0]
        sys.modules["antenv.axon_hooks"] = _mod
        antenv.axon_hooks = _mod
        from trn_agent_boot.trn_boot import _ntff_profile_via_ctypes
        _mod.set_axon_ntff_profile_hook(
            _ntff_profile_via_ctypes("/opt/axon/libaxon_pjrt.so")
        )
except Exception:
    pass

import concourse.bass as bass
import concourse.mybir as mybir
import concourse.tile as tile
from concourse import bacc
from concourse.bass_utils import run_bass_kernel_spmd
from concourse.masks import make_identity

F32 = mybir.dt.float32
F32R = mybir.dt.float32r
import ml_dtypes
MM_BF16 = True
MMDT = mybir.dt.bfloat16 if MM_BF16 else F32R
NPDT = ml_dtypes.bfloat16 if MM_BF16 else np.float32

T, B, D, A, H = 101, 512, 64, 12, 512
CONTEXT = 10
NCORES = 8
BL = B // NCORES          # 64 batch rows per core
NSTEPS_FULL = T - 1       # 100
G4 = 4 * H                # 2048

# gate chunk order: 0=g (tanh), 1=i, 2=f, 3=o (sigmoid)
SIGF = mybir.ActivationFunctionType.Sigmoid
TANHF = mybir.ActivationFunctionType.Tanh


def build(n_steps=NSTEPS_FULL):
    nc = bacc.Bacc("TRN2", target_bir_lowering=False, debug=False,
                   num_devices=NCORES)

    dp = lambda name, shape, dt=MMDT: nc.dram_tensor(
        name, shape, dt, kind="ExternalInput").ap()

    d = {
        "wih1": dp("wih1", [65, G4]),
        "whh1": dp("whh1", [4, 128, G4]),
        "wih2h": dp("wih2h", [4, 128, G4]),
        "wih2t": dp("wih2t", [97, G4]),
        "whh2": dp("whh2", [4, 128, G4]),
        "w1h": dp("w1h", [4, 128, H]),
        "w1t": dp("w1t", [65, H]),
        "w2": dp("w2", [4, 128, D]),
        "b2": dp("b2", [64, 1], F32),
        "tacT": dp("tacT", [65, CONTEXT, BL]),
        "tiledT": dp("tiledT", [NSTEPS_FULL, 97, BL]),
        "targT": dp("targT", [NSTEPS_FULL, D, BL], F32),
    }
    n_out = max(n_steps - (CONTEXT - 1), 0)
    d["outs"] = nc.dram_tensor("outs", [max(n_out, 1), D, BL], F32,
                               kind="ExternalOutput").ap()
    d["mae"] = nc.dram_tensor("mae", [D, 1], F32,
                              kind="ExternalOutput").ap()

    with tile.TileContext(nc) as tc:
        _body(nc, tc, n_steps, d)
    nc.finalize()
    return nc


def _body(nc, tc, n_steps, d):
    from contextlib import ExitStack
    ctx = ExitStack()
    wp = ctx.enter_context(tc.tile_pool(name="wp", bufs=1))
    st = ctx.enter_context(tc.tile_pool(name="st", bufs=1))
    wk = ctx.enter_context(tc.tile_pool(name="wk", bufs=2))
    sm = ctx.enter_context(tc.tile_pool(name="sm", bufs=3))
    gps = ctx.enter_context(tc.tile_pool(name="gps", bufs=4, space="PSUM"))
    tps = ctx.enter_context(tc.tile_pool(name="tps", bufs=2, space="PSUM"))
    fps = ctx.enter_context(tc.tile_pool(name="fps", bufs=2, space="PSUM"))

    r = lambda ap: ap.bitcast(MMDT) if not MM_BF16 else ap

    # ---- resident weights (f32r via direct DMA) ----
    wih1 = wp.tile([65, G4], MMDT)
    nc.sync.dma_start(out=wih1, in_=r(d["wih1"]))
    whh1 = wp.tile([128, 4, G4], MMDT)
    wih2h = wp.tile([128, 4, G4], MMDT)
    whh2 = wp.tile([128, 4, G4], MMDT)
    w1h = wp.tile([128, 4, H], MMDT)
    w2 = wp.tile([128, 4, D], MMDT)
    for k in range(4):
        nc.sync.dma_start(out=whh1[:, k, :], in_=r(d["whh1"][k]))
        nc.sync.dma_start(out=wih2h[:, k, :], in_=r(d["wih2h"][k]))
        nc.sync.dma_start(out=whh2[:, k, :], in_=r(d["whh2"][k]))
        nc.sync.dma_start(out=w1h[:, k, :], in_=r(d["w1h"][k]))
        nc.sync.dma_start(out=w2[:, k, :], in_=r(d["w2"][k]))
    wih2t = wp.tile([97, G4], MMDT)
    nc.sync.dma_start(out=wih2t, in_=r(d["wih2t"]))
    w1t = wp.tile([65, H], MMDT)
    nc.sync.dma_start(out=w1t, in_=r(d["w1t"]))
    b2 = wp.tile([64, 1], F32)
    nc.sync.dma_start(out=b2, in_=d["b2"])
    tacT = wp.tile([65, CONTEXT, BL], MMDT)
    nc.sync.dma_start(out=tacT, in_=r(d["tacT"]))
    ident = wp.tile([128, 128], F32)
    make_identity(nc, ident[:])

    # ---- persistent state ----
    c1 = st.tile([128, H], F32)
    c2 = st.tile([128, H], F32)
    h1T = st.tile([128, 4, BL], MMDT)
    h2T = st.tile([128, 4, BL], MMDT)
    inp1T = st.tile([65, BL], MMDT)
    out3T = st.tile([128, 4, BL], MMDT)
    acc = st.tile([D, 1], F32)
    nc.vector.memset(inp1T[64:65, :] if MM_BF16 else inp1T[64:65, :].bitcast(F32), 1.0)

    def cell_head(chunks, base, cstate, sig, tg, first):
        """ACT/DVE for gates g,i,f (chunk order 0=g 1=i 2=f 3=o); c update."""
        lo = slice(base, base + BL)
        nc.scalar.activation(tg[lo, :], chunks[0][lo, :], TANHF)
        nc.scalar.activation(sig[lo, 0:H], chunks[1][lo, :], SIGF)
        if first:
            nc.vector.tensor_mul(cstate[lo, :], sig[lo, 0:H], tg[lo, :])
        else:
            nc.scalar.activation(sig[lo, H:2 * H], chunks[2][lo, :], SIGF)
            t2 = wk.tile([128, H], F32, tag="t2")
            nc.vector.tensor_mul(t2[lo, :], sig[lo, 0:H], tg[lo, :])
            t1 = wk.tile([128, H], F32, tag="t1")
            nc.vector.tensor_mul(t1[lo, :], sig[lo, H:2 * H], cstate[lo, :])
            nc.vector.tensor_add(cstate[lo, :], t1[lo, :], t2[lo, :])

    def cell_tail_transpose(chunks, base, cstate, sig, hT):
        """sigmoid(o); then per 128-slice: tanh(c), h=o*tanh(c), PE-T, cast."""
        lo = slice(base, base + BL)
        nc.scalar.activation(sig[lo, 2 * H:3 * H], chunks[3][lo, :], SIGF)
        tcl = wk.tile([128, H], F32, tag="tcl")
        h = wk.tile([128, H], F32, tag="h")
        for k in range(4):
            sl = slice(128 * k, 128 * (k + 1))
            nc.scalar.activation(tcl[lo, sl], cstate[lo, sl], TANHF)
            nc.vector.tensor_mul(h[lo, sl], sig[lo, 2 * H + 128 * k:
                                                2 * H + 128 * (k + 1)],
                                 tcl[lo, sl])
            tp = tps.tile([128, BL], F32, tag="tp")
            nc.tensor.transpose(tp, h[lo, sl], ident[lo, lo])
            nc.vector.tensor_copy(hT[:, k, :], tp)

    NC = lambda n: slice(512 * n, 512 * (n + 1))
    LO = slice(0, BL)
    HI = slice(BL, 2 * BL)

    def pair(gtiles, cn, h1T_, h2T_, opens_lo, closes_lo):
        """col-tiled pairs: L1h (cols 0-63) || L2h (cols 64-127), chunk cn."""
        for k in range(4):
            nc.tensor.matmul(gtiles[cn][LO, :], h1T_[:, k, :],
                             whh1[:, k, NC(cn)],
                             start=(opens_lo and k == 0),
                             stop=(closes_lo and k == 3))
            nc.tensor.matmul(gtiles[cn][HI, :], h2T_[:, k, :],
                             whh2[:, k, NC(cn)], start=(k == 0), stop=False)

    gt = None       # gate chunk psums for step t (rows 0:64=L1, 64:128=L2)
    gt_next = None  # for step t+1, opened during step t's FC phase
    for t in range(n_steps):
        teacher = t < CONTEXT
        inp_stat = tacT[:, t, :] if teacher else inp1T[:]

        # (1) L1 x-part (+bias row): closes chunks g,i; opens chunks f,o
        if t == 0:
            gt = [gps.tile([128, H], F32, tag="gate", name=f"gt{t}_{n}")
                  for n in range(4)]
        for n in range(4):
            nc.tensor.matmul(gt[n][LO, :], inp_stat, wih1[:, NC(n)],
                             start=(t == 0 or n >= 2), stop=(t == 0 or n < 2))

        # (2) pairs for chunks f,o of THIS step fill the cell1 latency window
        if t > 0:
            pair(gt, 2, h1T, h2T, opens_lo=False, closes_lo=True)
            pair(gt, 3, h1T, h2T, opens_lo=False, closes_lo=True)

        # (3) L1 cell head (partitions 0:64) + tail/transpose -> h1T
        sig1 = wk.tile([128, 3 * H], F32, tag="sig1")
        tg1 = wk.tile([128, H], F32, tag="tg1")
        cell_head(gt, 0, c1, sig1, tg1, first=(t == 0))
        cell_tail_transpose(gt, 0, c1, sig1, h1T)

        # (4) L2 x2-part, k-major behind the h1T casts; tiled tail closes hi
        tiledT = sm.tile([97, BL], MMDT, tag="tiled")
        nc.sync.dma_start(out=tiledT, in_=r(d["tiledT"][t]))
        for k in range(4):
            for n in range(4):
                nc.tensor.matmul(gt[n][HI, :], h1T[:, k, :],
                                 wih2h[:, k, NC(n)],
                                 start=(t == 0 and k == 0), stop=False)
        for n in range(4):
            nc.tensor.matmul(gt[n][HI, :], tiledT[:], wih2t[:, NC(n)],
                             start=False, stop=True)

        # (5) FC1 inp1-part opens the FC1 group (independent of h2)
        fc1 = fps.tile([BL, H], F32, tag="fc")
        nc.tensor.matmul(fc1, inp_stat, w1t[:], start=True, stop=False)

        # (6) L2 cell (partitions 64:128) + tail/transpose -> h2T
        sig2 = wk.tile([128, 3 * H], F32, tag="sig2")
        tg2 = wk.tile([128, H], F32, tag="tg2")
        cell_head(gt, BL, c2, sig2, tg2, first=(t == 0))
        cell_tail_transpose(gt, BL, c2, sig2, h2T)

        # (7) FC1 h2-part closes the group
        for k in range(4):
            nc.tensor.matmul(fc1, h2T[:, k, :], w1h[:, k, :],
                             start=False, stop=(k == 3))

        # (8) pair chunk g for step t+1 runs while tanh(out3) drains
        if t + 1 < n_steps:
            gt_next = [gps.tile([128, H], F32, tag="gate",
                                name=f"gt{t + 1}_{n}") for n in range(4)]
            pair(gt_next, 0, h1T, h2T, opens_lo=True, closes_lo=False)
        out3 = wk.tile([BL, H], F32, tag="out3")
        nc.scalar.activation(out3, fc1, TANHF)

        # (9) out3 -> out3T, then pair chunk i while FC2 waits on casts
        for k in range(4):
            tp = tps.tile([128, BL], F32, tag="tp")
            nc.tensor.transpose(tp, out3[:, 128 * k:128 * (k + 1)],
                                ident[LO, LO])
            nc.vector.tensor_copy(out3T[:, k, :], tp)
        if t + 1 < n_steps:
            pair(gt_next, 1, h1T, h2T, opens_lo=True, closes_lo=False)

        # (10) FC2 feature-major: out4T = tanh(W2 @ out3T + b2)
        fc2 = fps.tile([D, BL], F32, tag="fc")
        for m in range(4):
            nc.tensor.matmul(fc2, w2[:, m, :], out3T[:, m, :],
                             start=(m == 0), stop=(m == 3))
        nc.scalar.activation(inp1T[0:D, :], fc2, TANHF, bias=b2[:, 0:1])
        out4f = wk.tile([D, BL], F32, tag="out4f")
        nc.scalar.activation(out4f, fc2, TANHF, bias=b2[:, 0:1])
        if t >= CONTEXT - 1:
            nc.gpsimd.dma_start(out=d["outs"][t - (CONTEXT - 1)], in_=out4f[:])

        # (11) mae
        targT = sm.tile([D, BL], F32, tag="targ")
        nc.sync.dma_start(out=targT, in_=d["targT"][t])
        diff = wk.tile([D, BL], F32, tag="diff")
        nc.vector.tensor_sub(diff, out4f[:], targT[:])
        if t == 0:
            nc.vector.tensor_reduce(acc[:], diff[:], mybir.AxisListType.X,
                                    mybir.AluOpType.add,
                                    apply_absolute_value=True)
        else:
            red = wk.tile([D, 1], F32, tag="red")
            nc.vector.tensor_reduce(red, diff[:], mybir.AxisListType.X,
                                    mybir.AluOpType.add,
                                    apply_absolute_value=True)
            nc.vector.tensor_add(acc[:], acc[:], red[:])
        gt = gt_next

    nc.gpsimd.dma_start(out=d["mae"], in_=acc[:])
    ctx.close()


# ---------------- host side ----------------

# gate chunk order [g i f o] applied to the 4H gate axis (torch order i,f,g,o)
_GPERM = np.concatenate([np.arange(1024, 1536), np.arange(0, 512),
                         np.arange(512, 1024), np.arange(1536, 2048)])


def prep_inputs(tactiles, actions, Wih1, Whh1, bih1, bhh1, Wih2, Whh2,
                bih2, bhh2, W1, b1, W2, b2):
    f = np.float32
    shared = {}
    wih1 = np.empty((65, G4), f)
    wih1[0:64] = Wih1.T[:, _GPERM]
    wih1[64] = (bih1 + bhh1)[_GPERM]
    shared["wih1"] = wih1
    shared["whh1"] = np.ascontiguousarray(
        Whh1.T[:, _GPERM].reshape(4, 128, G4))
    shared["wih2h"] = np.ascontiguousarray(
        Wih2.T[0:512][:, _GPERM].reshape(4, 128, G4))
    wih2t = np.empty((97, G4), f)
    wih2t[0:96] = Wih2.T[512:608][:, _GPERM]
    wih2t[96] = (bih2 + bhh2)[_GPERM]
    shared["wih2t"] = wih2t
    shared["whh2"] = np.ascontiguousarray(
        Whh2.T[:, _GPERM].reshape(4, 128, G4))
    shared["w1h"] = np.ascontiguousarray(W1.T[0:512].reshape(4, 128, H))
    w1t = np.empty((65, H), f)
    w1t[0:64] = W1.T[512:576]
    w1t[64] = b1
    shared["w1t"] = w1t
    shared["w2"] = np.ascontiguousarray(W2.T.reshape(4, 128, D))
    shared["b2"] = np.ascontiguousarray(b2.reshape(D, 1))

    state0 = actions[0]  # [B, A]
    in_maps = []
    for c in range(NCORES):
        bs = slice(c * BL, (c + 1) * BL)
        m = dict(shared)
        tacT = np.empty((65, CONTEXT, BL), f)
        for t in range(CONTEXT):
            tacT[0:64, t] = tactiles[t, bs].T
        tacT[64] = 1.0
        m["tacT"] = tacT
        tiledT = np.empty((NSTEPS_FULL, 97, BL), f)
        for t in range(NSTEPS_FULL):
            tiled = np.concatenate([actions[t + 1, bs], state0[bs]] * 4,
                                   axis=1)  # [BL, 96]
            tiledT[t, 0:96] = tiled.T
            tiledT[t, 96] = 1.0
        m["tiledT"] = tiledT
        m["targT"] = np.ascontiguousarray(
            tactiles[1:].transpose(0, 2, 1)[:, :, bs])
        keep_f32 = {"b2", "targT"}
        m = {k: (v if k in keep_f32 else v.astype(NPDT)) for k, v in m.items()}
        in_maps.append(m)
    return in_maps


_CACHE = {}


def run(inputs, n_steps=NSTEPS_FULL, trace=True):
    key = n_steps
    if key not in _CACHE:
        _CACHE[key] = build(n_steps)
    nc = _CACHE[key]
    in_maps = prep_inputs(**inputs)
    res = run_bass_kernel_spmd(nc, in_maps, core_ids=list(range(NCORES)),
                               trace=trace)
    n_out = max(n_steps - (CONTEXT - 1), 0)
    outs = np.concatenate(
        [res.results[c]["outs"][:n_out].transpose(0, 2, 1)
         for c in range(NCORES)], axis=1)  # [n_out, B, D]
    mae_sum = sum(float(res.results[c]["mae"].sum()) for c in range(NCORES))
    mae = np.float32(mae_sum / (B * D) / NSTEPS_FULL)
    return (mae, outs), res


def kernel(**inputs):
    (mae, outs), _ = run(inputs, trace=False)
    return mae, outs


# revision 8
# speedup vs baseline: 1.1382x; 1.1382x over previous
"""Trainium2 Bass kernel for the 2-layer-LSTM tactile rollout (nn_ACTP).

Strategy: data-parallel over batch (B=512 -> 64 rows/core on 8 cores).
All weights stay SBUF-resident (f32r, DMA'd directly). LSTM + FC1
matmuls run batch-major (activations stationary, weights moving,
float32r at N=512 -> full PE rate); FC2 runs feature-major (weights
stationary) so the rollout feedback path needs no extra transpose.
Gate order is reshuffled to [g i f o]; each gate gets its own [64,512]
PSUM bank so activations consume chunks as the PE closes them and the
next step's recurrence matmuls start early. Gate/FC1 biases ride a
ones-row in the stationary operand; the FC2 bias uses the ACT
per-partition bias port. h1/h2/out3 are re-transposed each step via
PE-transpose (4x [64,128] each). mae partials exit as a [64,1]
per-core column, reduced on host.
"""
import sys
import types

sys.path.insert(0, "/opt/trn_rl_repo")

import numpy as np

# ---- NTFF profile hook (missing antenv.axon_hooks in this image) ----
try:
    import antenv
    if "antenv.axon_hooks" not in sys.modules:
        _mod = types.ModuleType("antenv.axon_hooks")
        _hooks = [None]
        _mod.set_axon_ntff_profile_hook = lambda h: _hooks.__setitem__(0, h)
        _mod.get_axon_ntff_profile_hook = lambda: _hooks[0]
        sys.modules["antenv.axon_hooks"] = _mod
        antenv.axon_hooks = _mod
        from trn_agent_boot.trn_boot import _ntff_profile_via_ctypes
        _mod.set_axon_ntff_profile_hook(
            _ntff_profile_via_ctypes("/opt/axon/libaxon_pjrt.so")
        )
except Exception:
    pass

import concourse.bass as bass
import concourse.mybir as mybir
import concourse.tile as tile
from concourse import bacc
from concourse.bass_utils import run_bass_kernel_spmd
from concourse.masks import make_identity

F32 = mybir.dt.float32
F32R = mybir.dt.float32r
import ml_dtypes
MM_BF16 = True
MMDT = mybir.dt.bfloat16 if MM_BF16 else F32R
NPDT = ml_dtypes.bfloat16 if MM_BF16 else np.float32

T, B, D, A, H = 101, 512, 64, 12, 512
CONTEXT = 10
NCORES = 8
BL = B // NCORES          # 64 batch rows per core
NSTEPS_FULL = T - 1       # 100
G4 = 4 * H                # 2048

# gate chunk order: 0=g (tanh), 1=i, 2=f, 3=o (sigmoid)
SIGF = mybir.ActivationFunctionType.Sigmoid
TANHF = mybir.ActivationFunctionType.Tanh


def build(n_steps=NSTEPS_FULL):
    nc = bacc.Bacc("TRN2", target_bir_lowering=False, debug=False,
                   num_devices=NCORES)

    dp = lambda name, shape, dt=MMDT: nc.dram_tensor(
        name, shape, dt, kind="ExternalInput").ap()

    d = {
        "wih1": dp("wih1", [65, G4]),
        "whh1": dp("whh1", [4, 128, G4]),
        "wih2h": dp("wih2h", [4, 128, G4]),
        "wih2t": dp("wih2t", [97, G4]),
        "whh2": dp("whh2", [4, 128, G4]),
        "w1h": dp("w1h", [4, 128, H]),
        "w1t": dp("w1t", [65, H]),
        "w2": dp("w2", [4, 128, D]),
        "b2": dp("b2", [64, 1], F32),
        "tacT": dp("tacT", [65, CONTEXT, BL]),
        "tiledT": dp("tiledT", [NSTEPS_FULL, 97, BL]),
        "targT": dp("targT", [NSTEPS_FULL, D, BL], F32),
    }
    n_out = max(n_steps - (CONTEXT - 1), 0)
    d["outs"] = nc.dram_tensor("outs", [max(n_out, 1), D, BL], F32,
                               kind="ExternalOutput").ap()
    d["mae"] = nc.dram_tensor("mae", [D, 1], F32,
                              kind="ExternalOutput").ap()

    with tile.TileContext(nc) as tc:
        _body(nc, tc, n_steps, d)
    nc.finalize()
    return nc


def _body(nc, tc, n_steps, d):
    from contextlib import ExitStack
    ctx = ExitStack()
    wp = ctx.enter_context(tc.tile_pool(name="wp", bufs=1))
    st = ctx.enter_context(tc.tile_pool(name="st", bufs=1))
    wk = ctx.enter_context(tc.tile_pool(name="wk", bufs=2))
    sm = ctx.enter_context(tc.tile_pool(name="sm", bufs=3))
    gps = ctx.enter_context(tc.tile_pool(name="gps", bufs=4, space="PSUM"))
    tps = ctx.enter_context(tc.tile_pool(name="tps", bufs=2, space="PSUM"))
    fps = ctx.enter_context(tc.tile_pool(name="fps", bufs=2, space="PSUM"))

    r = lambda ap: ap.bitcast(MMDT) if not MM_BF16 else ap

    # ---- resident weights (f32r via direct DMA) ----
    wih1 = wp.tile([65, G4], MMDT)
    nc.sync.dma_start(out=wih1, in_=r(d["wih1"]))
    whh1 = wp.tile([128, 4, G4], MMDT)
    wih2h = wp.tile([128, 4, G4], MMDT)
    whh2 = wp.tile([128, 4, G4], MMDT)
    w1h = wp.tile([128, 4, H], MMDT)
    w2 = wp.tile([128, 4, D], MMDT)
    for k in range(4):
        nc.sync.dma_start(out=whh1[:, k, :], in_=r(d["whh1"][k]))
        nc.sync.dma_start(out=wih2h[:, k, :], in_=r(d["wih2h"][k]))
        nc.sync.dma_start(out=whh2[:, k, :], in_=r(d["whh2"][k]))
        nc.sync.dma_start(out=w1h[:, k, :], in_=r(d["w1h"][k]))
        nc.sync.dma_start(out=w2[:, k, :], in_=r(d["w2"][k]))
    wih2t = wp.tile([97, G4], MMDT)
    nc.sync.dma_start(out=wih2t, in_=r(d["wih2t"]))
    w1t = wp.tile([65, H], MMDT)
    nc.sync.dma_start(out=w1t, in_=r(d["w1t"]))
    b2 = wp.tile([64, 1], F32)
    nc.sync.dma_start(out=b2, in_=d["b2"])
    tacT = wp.tile([65, CONTEXT, BL], MMDT)
    nc.sync.dma_start(out=tacT, in_=r(d["tacT"]))
    ident = wp.tile([128, 128], F32)
    make_identity(nc, ident[:])

    # ---- persistent state ----
    c1 = st.tile([128, H], F32)
    c2 = st.tile([128, H], F32)
    h1T = st.tile([128, 4, BL], MMDT)
    h2T = st.tile([128, 4, BL], MMDT)
    inp1T = st.tile([65, BL], MMDT)
    out3T = st.tile([128, 4, BL], MMDT)
    acc = st.tile([D, 1], F32)
    nc.vector.memset(inp1T[64:65, :] if MM_BF16 else inp1T[64:65, :].bitcast(F32), 1.0)

    NC = lambda n: slice(512 * n, 512 * (n + 1))
    LO = slice(0, BL)
    HI = slice(BL, 2 * BL)

    def cell_head(chunks, base, cstate, sig, tg, first):
        """gate ACTs (2-slice pipelined) + c update. chunks: 0=g 1=i 2=f 3=o."""
        lo = slice(base, base + BL)
        HH = H // 2
        t2 = wk.tile([128, H], F32, tag="t2")
        t1 = wk.tile([128, H], F32, tag="t1")
        for j in range(2):
            sj = slice(HH * j, HH * (j + 1))
            nc.scalar.activation(tg[lo, sj], chunks[0][lo, sj], TANHF)
            nc.scalar.activation(sig[lo, sj], chunks[1][lo, sj], SIGF)
            if not first:
                nc.scalar.activation(sig[lo, H + HH * j:H + HH * (j + 1)],
                                     chunks[2][lo, sj], SIGF)
            if first:
                nc.vector.tensor_mul(cstate[lo, sj], sig[lo, sj], tg[lo, sj])
            else:
                nc.vector.tensor_mul(t2[lo, sj], sig[lo, sj], tg[lo, sj])
                nc.vector.tensor_mul(t1[lo, sj],
                                     sig[lo, H + HH * j:H + HH * (j + 1)],
                                     cstate[lo, sj])
                nc.vector.tensor_add(cstate[lo, sj], t1[lo, sj], t2[lo, sj])

    def cell_tail_transpose(chunks, base, cstate, sig, hT):
        """sigmoid(o); per 128-slice: tanh(c), h=o*tanh(c), PE-T, cast."""
        lo = slice(base, base + BL)
        nc.scalar.activation(sig[lo, 2 * H:3 * H], chunks[3][lo, :], SIGF)
        tcl = wk.tile([128, H], F32, tag="tcl")
        h = wk.tile([128, H], F32, tag="h")
        for k in range(4):
            sl = slice(128 * k, 128 * (k + 1))
            nc.scalar.activation(tcl[lo, sl], cstate[lo, sl], TANHF)
            nc.vector.tensor_mul(h[lo, sl], sig[lo, 2 * H + 128 * k:
                                                2 * H + 128 * (k + 1)],
                                 tcl[lo, sl])
            tp = tps.tile([128, BL], F32, tag="tp")
            nc.tensor.transpose(tp, h[lo, sl], ident[lo, lo])
            nc.vector.tensor_copy(hT[:, k, :], tp)

    def pair(gtiles, cn, h1T_, h2T_):
        """col-tiled pairs: L1h (cols 0-63) || L2h (cols 64-127), chunk cn."""
        for k in range(4):
            nc.tensor.matmul(gtiles[cn][LO, :], h1T_[:, k, :],
                             whh1[:, k, NC(cn)], start=(k == 0), stop=False)
            nc.tensor.matmul(gtiles[cn][HI, :], h2T_[:, k, :],
                             whh2[:, k, NC(cn)], start=(k == 0), stop=False)

    gt = None       # gate chunk psums for step t (rows 0:64=L1, 64:128=L2)
    gt_next = None  # for step t+1, filled during step t's FC phase
    for t in range(n_steps):
        teacher = t < CONTEXT
        inp_stat = tacT[:, t, :] if teacher else inp1T[:]

        # (1) L1 x-part (+bias row) closes all four lo chunks
        if t == 0:
            gt = [gps.tile([128, H], F32, tag="gate", name=f"gt{t}_{n}")
                  for n in range(4)]
        for n in range(4):
            nc.tensor.matmul(gt[n][LO, :], inp_stat, wih1[:, NC(n)],
                             start=(t == 0), stop=True)

        # (2) L2h chunk o fills the start of the cell1 window
        if t > 0:
            for k in range(4):
                nc.tensor.matmul(gt[3][HI, :], h2T[:, k, :],
                                 whh2[:, k, NC(3)], start=(k == 0),
                                 stop=False)

        # (3) L1 cell (partitions 0:64) -> h1T (pipelined slices)
        sig1 = wk.tile([128, 3 * H], F32, tag="sig1")
        tg1 = wk.tile([128, H], F32, tag="tg1")
        cell_head(gt, 0, c1, sig1, tg1, first=(t == 0))
        cell_tail_transpose(gt, 0, c1, sig1, h1T)

        # (4) L2 x2-part: k-major for k0-2 behind the h1T casts, then
        # chunk-serial [k3 + tiled tail] so cell2 starts per chunk early
        tiledT = sm.tile([97, BL], MMDT, tag="tiled")
        nc.sync.dma_start(out=tiledT, in_=r(d["tiledT"][t]))
        for k in range(3):
            for n in range(4):
                nc.tensor.matmul(gt[n][HI, :], h1T[:, k, :],
                                 wih2h[:, k, NC(n)],
                                 start=(t == 0 and k == 0), stop=False)
        for n in range(4):
            nc.tensor.matmul(gt[n][HI, :], h1T[:, 3, :], wih2h[:, 3, NC(n)],
                             start=False, stop=False)
            nc.tensor.matmul(gt[n][HI, :], tiledT[:], wih2t[:, NC(n)],
                             start=False, stop=True)

        # (5) FC1 inp1-part opens the FC1 group (independent of h2)
        fc1 = fps.tile([BL, H], F32, tag="fc")
        nc.tensor.matmul(fc1, inp_stat, w1t[:], start=True, stop=False)

        # (6) L2 cell (partitions 64:128) -> h2T
        sig2 = wk.tile([128, 3 * H], F32, tag="sig2")
        tg2 = wk.tile([128, H], F32, tag="tg2")
        cell_head(gt, BL, c2, sig2, tg2, first=(t == 0))
        cell_tail_transpose(gt, BL, c2, sig2, h2T)

        # (7) FC phase interleaved with next step's recurrence pairs
        for k in range(4):
            nc.tensor.matmul(fc1, h2T[:, k, :], w1h[:, k, :],
                             start=False, stop=(k == 3))
        if t + 1 < n_steps:
            gt_next = [gps.tile([128, H], F32, tag="gate",
                                name=f"gt{t + 1}_{n}") for n in range(4)]
            pair(gt_next, 0, h1T, h2T)
        out3 = wk.tile([BL, H], F32, tag="out3")
        nc.scalar.activation(out3, fc1, TANHF)
        for k in range(4):
            tp = tps.tile([128, BL], F32, tag="tp")
            nc.tensor.transpose(tp, out3[:, 128 * k:128 * (k + 1)],
                                ident[LO, LO])
            nc.vector.tensor_copy(out3T[:, k, :], tp)
        if t + 1 < n_steps:
            pair(gt_next, 1, h1T, h2T)
        fc2 = fps.tile([D, BL], F32, tag="fc")
        for m in range(4):
            nc.tensor.matmul(fc2, w2[:, m, :], out3T[:, m, :],
                             start=(m == 0), stop=(m == 3))
        if t + 1 < n_steps:
            pair(gt_next, 2, h1T, h2T)
            for k in range(4):
                nc.tensor.matmul(gt_next[3][LO, :], h1T[:, k, :],
                                 whh1[:, k, NC(3)], start=(k == 0),
                                 stop=False)
        nc.scalar.activation(inp1T[0:D, :], fc2, TANHF, bias=b2[:, 0:1])
        out4f = wk.tile([D, BL], F32, tag="out4f")
        nc.scalar.activation(out4f, fc2, TANHF, bias=b2[:, 0:1])
        if t >= CONTEXT - 1:
            nc.gpsimd.dma_start(out=d["outs"][t - (CONTEXT - 1)], in_=out4f[:])

        # (8) mae
        targT = sm.tile([D, BL], F32, tag="targ")
        nc.sync.dma_start(out=targT, in_=d["targT"][t])
        diff = wk.tile([D, BL], F32, tag="diff")
        nc.vector.tensor_sub(diff, out4f[:], targT[:])
        if t == 0:
            nc.vector.tensor_reduce(acc[:], diff[:], mybir.AxisListType.X,
                                    mybir.AluOpType.add,
                                    apply_absolute_value=True)
        else:
            red = wk.tile([D, 1], F32, tag="red")
            nc.vector.tensor_reduce(red, diff[:], mybir.AxisListType.X,
                                    mybir.AluOpType.add,
                                    apply_absolute_value=True)
            nc.vector.tensor_add(acc[:], acc[:], red[:])
        gt = gt_next

    nc.gpsimd.dma_start(out=d["mae"], in_=acc[:])
    ctx.close()


# ---------------- host side ----------------

# gate chunk order [g i f o] applied to the 4H gate axis (torch order i,f,g,o)
_GPERM = np.concatenate([np.arange(1024, 1536), np.arange(0, 512),
                         np.arange(512, 1024), np.arange(1536, 2048)])


def prep_inputs(tactiles, actions, Wih1, Whh1, bih1, bhh1, Wih2, Whh2,
                bih2, bhh2, W1, b1, W2, b2):
    f = np.float32
    shared = {}
    wih1 = np.empty((65, G4), f)
    wih1[0:64] = Wih1.T[:, _GPERM]
    wih1[64] = (bih1 + bhh1)[_GPERM]
    shared["wih1"] = wih1
    shared["whh1"] = np.ascontiguousarray(
        Whh1.T[:, _GPERM].reshape(4, 128, G4))
    shared["wih2h"] = np.ascontiguousarray(
        Wih2.T[0:512][:, _GPERM].reshape(4, 128, G4))
    wih2t = np.empty((97, G4), f)
    wih2t[0:96] = Wih2.T[512:608][:, _GPERM]
    wih2t[96] = (bih2 + bhh2)[_GPERM]
    shared["wih2t"] = wih2t
    shared["whh2"] = np.ascontiguousarray(
        Whh2.T[:, _GPERM].reshape(4, 128, G4))
    shared["w1h"] = np.ascontiguousarray(W1.T[0:512].reshape(4, 128, H))
    w1t = np.empty((65, H), f)
    w1t[0:64] = W1.T[512:576]
    w1t[64] = b1
    shared["w1t"] = w1t
    shared["w2"] = np.ascontiguousarray(W2.T.reshape(4, 128, D))
    shared["b2"] = np.ascontiguousarray(b2.reshape(D, 1))

    state0 = actions[0]  # [B, A]
    in_maps = []
    for c in range(NCORES):
        bs = slice(c * BL, (c + 1) * BL)
        m = dict(shared)
        tacT = np.empty((65, CONTEXT, BL), f)
        for t in range(CONTEXT):
            tacT[0:64, t] = tactiles[t, bs].T
        tacT[64] = 1.0
        m["tacT"] = tacT
        tiledT = np.empty((NSTEPS_FULL, 97, BL), f)
        for t in range(NSTEPS_FULL):
            tiled = np.concatenate([actions[t + 1, bs], state0[bs]] * 4,
                                   axis=1)  # [BL, 96]
            tiledT[t, 0:96] = tiled.T
            tiledT[t, 96] = 1.0
        m["tiledT"] = tiledT
        m["targT"] = np.ascontiguousarray(
            tactiles[1:].transpose(0, 2, 1)[:, :, bs])
        keep_f32 = {"b2", "targT"}
        m = {k: (v if k in keep_f32 else v.astype(NPDT)) for k, v in m.items()}
        in_maps.append(m)
    return in_maps


_CACHE = {}


def run(inputs, n_steps=NSTEPS_FULL, trace=True):
    key = n_steps
    if key not in _CACHE:
        _CACHE[key] = build(n_steps)
    nc = _CACHE[key]
    in_maps = prep_inputs(**inputs)
    res = run_bass_kernel_spmd(nc, in_maps, core_ids=list(range(NCORES)),
                               trace=trace)
    n_out = max(n_steps - (CONTEXT - 1), 0)
    outs = np.concatenate(
        [res.results[c]["outs"][:n_out].transpose(0, 2, 1)
         for c in range(NCORES)], axis=1)  # [n_out, B, D]
    mae_sum = sum(float(res.results[c]["mae"].sum()) for c in range(NCORES))
    mae = np.float32(mae_sum / (B * D) / NSTEPS_FULL)
    return (mae, outs), res


def kernel(**inputs):
    (mae, outs), _ = run(inputs, trace=False)
    return mae, outs


# revision 9
# speedup vs baseline: 1.7382x; 1.5271x over previous
"""Trainium2 Bass kernel for the 2-layer-LSTM tactile rollout (nn_ACTP).

Strategy: data-parallel over batch (B=512 -> 64 rows/core on 8 cores).
All weights stay SBUF-resident (f32r, DMA'd directly). LSTM + FC1
matmuls run batch-major (activations stationary, weights moving,
float32r at N=512 -> full PE rate); FC2 runs feature-major (weights
stationary) so the rollout feedback path needs no extra transpose.
Gate order is reshuffled to [g i f o]; each gate gets its own [64,512]
PSUM bank so activations consume chunks as the PE closes them and the
next step's recurrence matmuls start early. Gate/FC1 biases ride a
ones-row in the stationary operand; the FC2 bias uses the ACT
per-partition bias port. h1/h2/out3 are re-transposed each step via
PE-transpose (4x [64,128] each). mae partials exit as a [64,1]
per-core column, reduced on host.
"""
import sys
import types

sys.path.insert(0, "/opt/trn_rl_repo")

import numpy as np

# ---- NTFF profile hook (missing antenv.axon_hooks in this image) ----
try:
    import antenv
    if "antenv.axon_hooks" not in sys.modules:
        _mod = types.ModuleType("antenv.axon_hooks")
        _hooks = [None]
        _mod.set_axon_ntff_profile_hook = lambda h: _hooks.__setitem__(0, h)
        _mod.get_axon_ntff_profile_hook = lambda: _hooks[0]
        sys.modules["antenv.axon_hooks"] = _mod
        antenv.axon_hooks = _mod
        from trn_agent_boot.trn_boot import _ntff_profile_via_ctypes
        _mod.set_axon_ntff_profile_hook(
            _ntff_profile_via_ctypes("/opt/axon/libaxon_pjrt.so")
        )
except Exception:
    pass

import concourse.bass as bass
import concourse.mybir as mybir
import concourse.tile as tile
from concourse import bacc
from concourse.bass_utils import run_bass_kernel_spmd
from concourse.masks import make_identity

F32 = mybir.dt.float32
F32R = mybir.dt.float32r
import ml_dtypes
MM_BF16 = True
MMDT = mybir.dt.bfloat16 if MM_BF16 else F32R
NPDT = ml_dtypes.bfloat16 if MM_BF16 else np.float32

T, B, D, A, H = 101, 512, 64, 12, 512
CONTEXT = 10
NCORES = 8
BL = B // NCORES          # 64 batch rows per core
NSTEPS_FULL = T - 1       # 100
G4 = 4 * H                # 2048

# gate chunk order: 0=g (tanh), 1=i, 2=f, 3=o (sigmoid)
SIGF = mybir.ActivationFunctionType.Sigmoid
TANHF = mybir.ActivationFunctionType.Tanh


def build(n_steps=NSTEPS_FULL):
    nc = bacc.Bacc("TRN2", target_bir_lowering=False, debug=False,
                   num_devices=NCORES)

    dp = lambda name, shape, dt=MMDT: nc.dram_tensor(
        name, shape, dt, kind="ExternalInput").ap()

    d = {
        "wih1": dp("wih1", [65, G4]),
        "whh1": dp("whh1", [4, 128, G4]),
        "wih2h": dp("wih2h", [4, 128, G4]),
        "wih2t": dp("wih2t", [97, G4]),
        "whh2": dp("whh2", [4, 128, G4]),
        "w1h": dp("w1h", [4, 128, H]),
        "w1t": dp("w1t", [65, H]),
        "w2": dp("w2", [4, 128, D]),
        "b2": dp("b2", [64, 1], F32),
        "tacT": dp("tacT", [65, CONTEXT, BL]),
        "tiledT": dp("tiledT", [NSTEPS_FULL, 97, BL]),
        "targT": dp("targT", [NSTEPS_FULL, D, BL], F32),
    }
    n_out = max(n_steps - (CONTEXT - 1), 0)
    d["outs"] = nc.dram_tensor("outs", [max(n_out, 1), D, BL], F32,
                               kind="ExternalOutput").ap()
    d["mae"] = nc.dram_tensor("mae", [D, 1], F32,
                              kind="ExternalOutput").ap()

    with tile.TileContext(nc) as tc:
        _body(nc, tc, n_steps, d)
    nc.finalize()
    return nc


def _body(nc, tc, n_steps, d):
    from contextlib import ExitStack
    ctx = ExitStack()
    wp = ctx.enter_context(tc.tile_pool(name="wp", bufs=1))
    st = ctx.enter_context(tc.tile_pool(name="st", bufs=1))
    wk = ctx.enter_context(tc.tile_pool(name="wk", bufs=2))
    sm = ctx.enter_context(tc.tile_pool(name="sm", bufs=3))
    gps = ctx.enter_context(tc.tile_pool(name="gps", bufs=4, space="PSUM"))
    tps = ctx.enter_context(tc.tile_pool(name="tps", bufs=2, space="PSUM"))
    fps = ctx.enter_context(tc.tile_pool(name="fps", bufs=2, space="PSUM"))

    r = lambda ap: ap.bitcast(MMDT) if not MM_BF16 else ap

    # ---- resident weights (f32r via direct DMA) ----
    wih1 = wp.tile([65, G4], MMDT)
    nc.sync.dma_start(out=wih1, in_=r(d["wih1"]))
    whh1 = wp.tile([128, 4, G4], MMDT)
    wih2h = wp.tile([128, 4, G4], MMDT)
    whh2 = wp.tile([128, 4, G4], MMDT)
    w1h = wp.tile([128, 4, H], MMDT)
    w2 = wp.tile([128, 4, D], MMDT)
    for k in range(4):
        nc.sync.dma_start(out=whh1[:, k, :], in_=r(d["whh1"][k]))
        nc.sync.dma_start(out=wih2h[:, k, :], in_=r(d["wih2h"][k]))
        nc.sync.dma_start(out=whh2[:, k, :], in_=r(d["whh2"][k]))
        nc.sync.dma_start(out=w1h[:, k, :], in_=r(d["w1h"][k]))
        nc.sync.dma_start(out=w2[:, k, :], in_=r(d["w2"][k]))
    wih2t = wp.tile([97, G4], MMDT)
    nc.sync.dma_start(out=wih2t, in_=r(d["wih2t"]))
    w1t = wp.tile([65, H], MMDT)
    nc.sync.dma_start(out=w1t, in_=r(d["w1t"]))
    b2 = wp.tile([64, 1], F32)
    nc.sync.dma_start(out=b2, in_=d["b2"])
    tacT = wp.tile([65, CONTEXT, BL], MMDT)
    nc.sync.dma_start(out=tacT, in_=r(d["tacT"]))
    ident = wp.tile([128, 128], F32)
    make_identity(nc, ident[:])

    # ---- persistent state ----
    c1 = st.tile([128, H // 2], F32)
    c2 = st.tile([128, H // 2], F32)
    h1T = st.tile([128, 4, BL], MMDT)
    h2T = st.tile([128, 4, BL], MMDT)
    inp1T = st.tile([65, BL], MMDT)
    out3T = st.tile([128, 4, BL], MMDT)
    acc = st.tile([D, 1], F32)
    nc.vector.memset(inp1T[64:65, :] if MM_BF16 else inp1T[64:65, :].bitcast(F32), 1.0)

    # ---- folded-H layout ----------------------------------------------
    # Every [64, 512] gate chunk is computed as twin col-tiled matmuls of
    # N=256: gate cols [0:256) land on psum partitions 0-63, cols [256:512)
    # on partitions 64-127 ("folded"). The cell elementwise then runs on
    # [128, 256] tiles at full lane utilization, and the PE-transposes
    # unfold back to feature-major h1T/h2T for free.
    HF = H // 2  # 256
    LOQ = slice(0, BL)
    HIQ = slice(BL, 2 * BL)

    def twin(tile_, off, stat, w, colbase, start, stop):
        nc.tensor.matmul(tile_[LOQ, off:off + HF], stat,
                         w[:, colbase:colbase + HF], start=start, stop=stop)
        nc.tensor.matmul(tile_[HIQ, off:off + HF], stat,
                         w[:, colbase + HF:colbase + 2 * HF],
                         start=start, stop=stop)

    def gchunk(gtiles, n):
        return gtiles[n // 2][:, (n % 2) * HF:(n % 2) * HF + HF]

    def cell_head(gtiles, cstate, sig, tg, first):
        """folded gate ACTs + c update; chunk order 0=g 1=i 2=f 3=o."""
        nc.scalar.activation(tg[:], gchunk(gtiles, 0), TANHF)
        nc.scalar.activation(sig[:, 0:HF], gchunk(gtiles, 1), SIGF)
        if first:
            nc.vector.tensor_mul(cstate[:], sig[:, 0:HF], tg[:])
        else:
            nc.scalar.activation(sig[:, HF:2 * HF], gchunk(gtiles, 2), SIGF)
            t2 = wk.tile([128, HF], F32, tag="t2")
            nc.vector.tensor_mul(t2, sig[:, 0:HF], tg[:])
            t1 = wk.tile([128, HF], F32, tag="t1")
            nc.vector.tensor_mul(t1, sig[:, HF:2 * HF], cstate[:])
            nc.vector.tensor_add(cstate[:], t1[:], t2[:])

    def cell_tail_transpose(gtiles, cstate, sig, hT):
        """sigmoid(o); per 128-col slice: tanh(c), h=o*tanh(c), two
        PE-transposes (lo quadrant -> k=s, hi quadrant -> k=s+2), casts."""
        nc.scalar.activation(sig[:, 2 * HF:3 * HF], gchunk(gtiles, 3), SIGF)
        tcl = wk.tile([128, HF], F32, tag="tcl")
        h = wk.tile([128, HF], F32, tag="h")
        for sft in range(2):
            sl = slice(128 * sft, 128 * (sft + 1))
            nc.scalar.activation(tcl[:, sl], cstate[:, sl], TANHF)
            nc.vector.tensor_mul(h[:, sl], sig[:, 2 * HF + 128 * sft:
                                               2 * HF + 128 * (sft + 1)],
                                 tcl[:, sl])
            tp = tps.tile([128, BL], F32, tag="tp")
            nc.tensor.transpose(tp, h[LOQ, sl], ident[LOQ, LOQ])
            nc.vector.tensor_copy(hT[:, sft, :], tp)
            tp2 = tps.tile([128, BL], F32, tag="tp")
            nc.tensor.transpose(tp2, h[HIQ, sl], ident[HIQ, HIQ])
            nc.vector.tensor_copy(hT[:, sft + 2, :], tp2)

    gl1 = None   # L1 gate psum tiles for step t: [g|i], [f|o], folded
    for t in range(n_steps):
        teacher = t < CONTEXT
        inp_stat = tacT[:, t, :] if teacher else inp1T[:]

        # (1) L1 x-part (+bias row) closes all lo/hi chunk groups
        if t == 0:
            gl1 = [gps.tile([128, 2 * HF], F32, tag="gate",
                            name=f"gl1_{t}_{j}") for j in range(2)]
        for n in range(4):
            twin(gl1[n // 2], (n % 2) * HF, inp_stat, wih1, 512 * n,
                 start=(t == 0), stop=True)

        # (2) L2 h-part fills the cell1 window
        gl2 = [gps.tile([128, 2 * HF], F32, tag="gate",
                        name=f"gl2_{t}_{j}") for j in range(2)]
        if t > 0:
            for n in range(4):
                for k in range(4):
                    twin(gl2[n // 2], (n % 2) * HF, h2T[:, k, :], whh2[:, k, :],
                         512 * n, start=(k == 0), stop=False)

        # (3) L1 cell (folded) -> h1T
        sig1 = wk.tile([128, 3 * HF], F32, tag="sig1")
        tg1 = wk.tile([128, HF], F32, tag="tg1")
        cell_head(gl1, c1, sig1, tg1, first=(t == 0))
        cell_tail_transpose(gl1, c1, sig1, h1T)

        # (4) L2 x2-part: k-major behind the h1T casts, then close chunks
        tiledT = sm.tile([97, BL], MMDT, tag="tiled")
        nc.sync.dma_start(out=tiledT, in_=r(d["tiledT"][t]))
        for k in range(3):
            for n in range(4):
                twin(gl2[n // 2], (n % 2) * HF, h1T[:, k, :], wih2h[:, k, :],
                     512 * n, start=(t == 0 and k == 0), stop=False)
        for n in range(4):
            twin(gl2[n // 2], (n % 2) * HF, h1T[:, 3, :], wih2h[:, 3, :],
                 512 * n, start=False, stop=False)
            twin(gl2[n // 2], (n % 2) * HF, tiledT[:], wih2t, 512 * n,
                 start=False, stop=True)

        # (5) FC1 inp1-part opens the (folded) FC1 group early
        fc1 = fps.tile([128, HF], F32, tag="fc")
        nc.tensor.matmul(fc1[LOQ, :], inp_stat, w1t[:, 0:HF],
                         start=True, stop=False)
        nc.tensor.matmul(fc1[HIQ, :], inp_stat, w1t[:, HF:2 * HF],
                         start=True, stop=False)

        # (6) L2 cell (folded) -> h2T
        sig2 = wk.tile([128, 3 * HF], F32, tag="sig2")
        tg2 = wk.tile([128, HF], F32, tag="tg2")
        cell_head(gl2, c2, sig2, tg2, first=(t == 0))
        cell_tail_transpose(gl2, c2, sig2, h2T)

        # (7) FC phase interleaved with next step's L1 recurrence
        for k in range(4):
            nc.tensor.matmul(fc1[LOQ, :], h2T[:, k, :], w1h[:, k, 0:HF],
                             start=False, stop=(k == 3))
            nc.tensor.matmul(fc1[HIQ, :], h2T[:, k, :], w1h[:, k, HF:2 * HF],
                             start=False, stop=(k == 3))
        if t + 1 < n_steps:
            gl1 = [gps.tile([128, 2 * HF], F32, tag="gate",
                            name=f"gl1_{t + 1}_{j}") for j in range(2)]
            for n in range(2):
                for k in range(4):
                    twin(gl1[n // 2], (n % 2) * HF, h1T[:, k, :],
                         whh1[:, k, :], 512 * n, start=(k == 0), stop=False)
        out3 = wk.tile([128, HF], F32, tag="out3")
        nc.scalar.activation(out3, fc1, TANHF)
        for sft in range(2):
            sl = slice(128 * sft, 128 * (sft + 1))
            tp = tps.tile([128, BL], F32, tag="tp")
            nc.tensor.transpose(tp, out3[LOQ, sl], ident[LOQ, LOQ])
            nc.vector.tensor_copy(out3T[:, sft, :], tp)
            tp2 = tps.tile([128, BL], F32, tag="tp")
            nc.tensor.transpose(tp2, out3[HIQ, sl], ident[HIQ, HIQ])
            nc.vector.tensor_copy(out3T[:, sft + 2, :], tp2)
        if t + 1 < n_steps:
            for n in range(2, 4):
                for k in range(4):
                    twin(gl1[n // 2], (n % 2) * HF, h1T[:, k, :],
                         whh1[:, k, :], 512 * n, start=(k == 0), stop=False)
        fc2 = fps.tile([D, BL], F32, tag="fc")
        for m in range(4):
            nc.tensor.matmul(fc2, w2[:, m, :], out3T[:, m, :],
                             start=(m == 0), stop=(m == 3))
        nc.scalar.activation(inp1T[0:D, :], fc2, TANHF, bias=b2[:, 0:1])
        out4f = wk.tile([D, BL], F32, tag="out4f")
        nc.scalar.activation(out4f, fc2, TANHF, bias=b2[:, 0:1])
        if t >= CONTEXT - 1:
            nc.gpsimd.dma_start(out=d["outs"][t - (CONTEXT - 1)], in_=out4f[:])

        # (8) mae
        targT = sm.tile([D, BL], F32, tag="targ")
        nc.sync.dma_start(out=targT, in_=d["targT"][t])
        diff = wk.tile([D, BL], F32, tag="diff")
        nc.vector.tensor_sub(diff, out4f[:], targT[:])
        if t == 0:
            nc.vector.tensor_reduce(acc[:], diff[:], mybir.AxisListType.X,
                                    mybir.AluOpType.add,
                                    apply_absolute_value=True)
        else:
            red = wk.tile([D, 1], F32, tag="red")
            nc.vector.tensor_reduce(red, diff[:], mybir.AxisListType.X,
                                    mybir.AluOpType.add,
                                    apply_absolute_value=True)
            nc.vector.tensor_add(acc[:], acc[:], red[:])

    nc.gpsimd.dma_start(out=d["mae"], in_=acc[:])
    ctx.close()


# ---------------- host side ----------------

# gate chunk order [g i f o] applied to the 4H gate axis (torch order i,f,g,o)
_GPERM = np.concatenate([np.arange(1024, 1536), np.arange(0, 512),
                         np.arange(512, 1024), np.arange(1536, 2048)])


def prep_inputs(tactiles, actions, Wih1, Whh1, bih1, bhh1, Wih2, Whh2,
                bih2, bhh2, W1, b1, W2, b2):
    f = np.float32
    shared = {}
    wih1 = np.empty((65, G4), f)
    wih1[0:64] = Wih1.T[:, _GPERM]
    wih1[64] = (bih1 + bhh1)[_GPERM]
    shared["wih1"] = wih1
    shared["whh1"] = np.ascontiguousarray(
        Whh1.T[:, _GPERM].reshape(4, 128, G4))
    shared["wih2h"] = np.ascontiguousarray(
        Wih2.T[0:512][:, _GPERM].reshape(4, 128, G4))
    wih2t = np.empty((97, G4), f)
    wih2t[0:96] = Wih2.T[512:608][:, _GPERM]
    wih2t[96] = (bih2 + bhh2)[_GPERM]
    shared["wih2t"] = wih2t
    shared["whh2"] = np.ascontiguousarray(
        Whh2.T[:, _GPERM].reshape(4, 128, G4))
    shared["w1h"] = np.ascontiguousarray(W1.T[0:512].reshape(4, 128, H))
    w1t = np.empty((65, H), f)
    w1t[0:64] = W1.T[512:576]
    w1t[64] = b1
    shared["w1t"] = w1t
    shared["w2"] = np.ascontiguousarray(W2.T.reshape(4, 128, D))
    shared["b2"] = np.ascontiguousarray(b2.reshape(D, 1))

    state0 = actions[0]  # [B, A]
    in_maps = []
    for c in range(NCORES):
        bs = slice(c * BL, (c + 1) * BL)
        m = dict(shared)
        tacT = np.empty((65, CONTEXT, BL), f)
        for t in range(CONTEXT):
            tacT[0:64, t] = tactiles[t, bs].T
        tacT[64] = 1.0
        m["tacT"] = tacT
        tiledT = np.empty((NSTEPS_FULL, 97, BL), f)
        for t in range(NSTEPS_FULL):
            tiled = np.concatenate([actions[t + 1, bs], state0[bs]] * 4,
                                   axis=1)  # [BL, 96]
            tiledT[t, 0:96] = tiled.T
            tiledT[t, 96] = 1.0
        m["tiledT"] = tiledT
        m["targT"] = np.ascontiguousarray(
            tactiles[1:].transpose(0, 2, 1)[:, :, bs])
        keep_f32 = {"b2", "targT"}
        m = {k: (v if k in keep_f32 else v.astype(NPDT)) for k, v in m.items()}
        in_maps.append(m)
    return in_maps


_CACHE = {}


def run(inputs, n_steps=NSTEPS_FULL, trace=True):
    key = n_steps
    if key not in _CACHE:
        _CACHE[key] = build(n_steps)
    nc = _CACHE[key]
    in_maps = prep_inputs(**inputs)
    res = run_bass_kernel_spmd(nc, in_maps, core_ids=list(range(NCORES)),
                               trace=trace)
    n_out = max(n_steps - (CONTEXT - 1), 0)
    outs = np.concatenate(
        [res.results[c]["outs"][:n_out].transpose(0, 2, 1)
         for c in range(NCORES)], axis=1)  # [n_out, B, D]
    mae_sum = sum(float(res.results[c]["mae"].sum()) for c in range(NCORES))
    mae = np.float32(mae_sum / (B * D) / NSTEPS_FULL)
    return (mae, outs), res


def kernel(**inputs):
    (mae, outs), _ = run(inputs, trace=False)
    return mae, outs
